# revision 16
# baseline (speedup 1.0000x reference)
"""AWQ quantized linear (4096 -> 11008) on 8 trn2 NeuronCores.

Column-parallel sharding: each core owns OUT/8 = 1376 output features.
Host side does only sharding + index-permutation (no arithmetic): the packed
int32 words are resharded chunk-major ([NK, 128, 11*64] per core, padded to
11 full 128-row o-tiles) so every device DMA is contiguous per partition.

Per core (all compute on device):
  - unpack int32-packed nibbles (low/high interleaved) on DVE
  - per-group (128-wide) affine dequant via tensor_scalar (per-partition
    scalars), chunk-outer so wt[:, c, :] completes incrementally
  - PE-transpose weight tiles to [IN, OUT_SH] fp16 resident in SBUF, folding
    the per-input-channel inv_scale into the PSUM->SBUF copy
  - x path: SWDGE cast-DMA f32->fp16 into per-block internal DRAM tiles, then
    hardware DMA-transpose (xbar) loads xT k-chunk tiles straight into SBUF
  - DMA queue separation: casts on gpsimd (SWDGE), xbar transposes on sync
    (HWDGE-SP), everything else on scalar (HWDGE-ACT)
  - fp16 matmuls accumulate in f32 PSUM over 32 K-chunks; N-slices 512/512/352
  - bias added during PSUM->SBUF copy; f32 stores
"""

import sys

for _p in ("/opt/trn_rl_repo", "/opt/pypackages"):
    if _p not in sys.path:
        sys.path.append(_p)

import numpy as np

import concourse.bass as bass
import concourse.mybir as mybir
import concourse.tile as tile
from concourse import bacc
from concourse.bass_utils import run_bass_kernel_spmd
from concourse.masks import make_identity

IN = 4096
OUT = 11008
N_CORES = 8
OUT_SH = OUT // N_CORES  # 1376
T = 8192
NK = IN // 128  # 32 k-chunks
P = 128
TB = 512       # token block for DMA-transpose staging

dt = mybir.dt
Alu = mybir.AluOpType
Act = mybir.ActivationFunctionType


def build(n_t_tiles=T // P, out_sh=OUT_SH):
    n_o_tiles = (out_sh + P - 1) // P          # 11
    out_pad = n_o_tiles * P                    # 1408 (wt padded, output not)
    nsl = []
    n0 = 0
    while n0 < out_sh:
        nsz = min(512, out_sh - n0)
        nsl.append((n0, nsz))
        n0 += nsz

    n_tok = n_t_tiles * P
    tb = min(TB, n_tok)
    n_blocks = (n_tok + tb - 1) // tb
    tiles_per_block = tb // P

    nc = bacc.Bacc("TRN2", target_bir_lowering=False, debug=False,
                   num_devices=N_CORES)
    xt = nc.dram_tensor("xt", [IN, n_tok], dt.float32,
                        kind="ExternalInput").ap()
    # host-repacked: pk_h[c, p, ot*64+f] = packed[(k*OUT_SH)+ot*128+p, c*64+f]
    pk = nc.dram_tensor("pk", [NK, P, n_o_tiles * 64], dt.int32,
                        kind="ExternalInput").ap()
    sc = nc.dram_tensor("sc", [P, n_o_tiles, NK], dt.float32,
                        kind="ExternalInput").ap()
    of = nc.dram_tensor("of", [P, n_o_tiles, NK], dt.float32,
                        kind="ExternalInput").ap()
    inv = nc.dram_tensor("inv", [P, NK], dt.float32,
                         kind="ExternalInput").ap()
    bias = nc.dram_tensor("bias", [out_sh], dt.float32,
                          kind="ExternalInput").ap()
    out = nc.dram_tensor("out", [n_tok, out_sh], dt.float32,
                         kind="ExternalOutput").ap()

    with tile.TileContext(nc) as tc:
        with (
            tc.tile_pool(name="const", bufs=1) as constp,
            tc.tile_pool(name="wtp", bufs=1) as wtp,
            tc.tile_pool(name="prep", bufs=2) as prep,
            tc.tile_pool(name="prepsm", bufs=2) as prepsm,
            tc.tile_pool(name="xtp", bufs=2) as xtp,
            tc.tile_pool(name="outp", bufs=2) as outp,
            tc.tile_pool(name="pmm", bufs=2 * len(nsl), space="PSUM") as pmm,
            tc.tile_pool(name="ptp", bufs=2, space="PSUM") as ptp,
        ):
            ident16 = constp.tile([P, P], dt.float16)
            make_identity(nc, ident16[:])

            inv_sb = constp.tile([P, NK], dt.float32)
            nc.scalar.dma_start(inv_sb[:], inv[:])

            bias_bc = constp.tile([P, out_sh], dt.float16)
            nc.gpsimd.dma_start(bias_bc[:],
                                bias[None, :].to_broadcast([P, out_sh]))

            # fp16 W^T, laid out [128 (i within chunk), NK chunks, out_pad]
            wt = wtp.tile([P, NK, out_pad], dt.float16)

            # ---- weight prep (chunk-outer; all loads contiguous) ----
            sc_all = constp.tile([P, n_o_tiles, NK], dt.float32)
            of_all = constp.tile([P, n_o_tiles, NK], dt.float32)
            nc.scalar.dma_start(sc_all[:], sc[:])
            nc.scalar.dma_start(of_all[:], of[:])

            for c in range(NK):
                pkt = prep.tile([P, n_o_tiles * 64], dt.int32, tag="pkt",
                                name=f"pkt{c}")
                nc.scalar.dma_start(pkt[:], pk[c])
                wq = prepsm.tile([P, n_o_tiles * 64, 2], dt.int32, tag="wq",
                                 name=f"wq{c}")
                nc.vector.tensor_scalar(wq[:, :, 0], pkt[:], 15, None,
                                        op0=Alu.bitwise_and)
                nc.vector.tensor_scalar(wq[:, :, 1], pkt[:], 4, None,
                                        op0=Alu.logical_shift_right)
                for ot in range(n_o_tiles):
                    o0 = ot * P
                    wd = prepsm.tile([P, P], dt.float16, tag="wd")
                    wq_ot = wq[:, ot * 64:(ot + 1) * 64, :].rearrange(
                        "p a b -> p (a b)")
                    if ot % 2 == 0:
                        nc.vector.tensor_scalar(
                            wd[:], wq_ot,
                            sc_all[:, ot, c:c + 1], of_all[:, ot, c:c + 1],
                            op0=Alu.mult, op1=Alu.add)
                    else:
                        nc.scalar.activation(
                            wd[:], wq_ot, Act.Identity,
                            bias=of_all[:, ot, c:c + 1],
                            scale=sc_all[:, ot, c:c + 1])
                    ps = ptp.tile([P, P], dt.float32, tag="tp")
                    # transpose as a normal fp16 matmul against identity:
                    # exact (x*1.0 into f32 PSUM) and keeps the HAM clock warm
                    nc.tensor.matmul(ps[:], lhsT=wd[:], rhs=ident16[:],
                                     start=True, stop=True)
                    if ot % 2 == 0:
                        nc.scalar.activation(wt[:, c, o0:o0 + P], ps[:],
                                             Act.Copy,
                                             scale=inv_sb[:, c:c + 1])
                    else:
                        nc.vector.tensor_scalar(wt[:, c, o0:o0 + P], ps[:],
                                                inv_sb[:, c:c + 1], None,
                                                op0=Alu.mult)

            # ---- main loop over token blocks ----
            for b in range(n_blocks):
                xtb = xtp.tile([P, NK, tb], dt.float16, tag="xtb",
                               name=f"xtb{b}")
                for c in range(NK):
                    nc.gpsimd.dma_start(
                        xtb[:, c, :],
                        xt[c * P:(c + 1) * P, b * tb:(b + 1) * tb])
                for it in range(tiles_per_block):
                    tt = b * tiles_per_block + it
                    t0 = tt * P
                    po = [pmm.tile([P, 512], dt.float32, tag="po",
                                   name=f"po{tt}_{j}")
                          for j in range(len(nsl))]
                    for c in range(NK):
                        for j, (n0, nsz) in enumerate(nsl):
                            nc.tensor.matmul(
                                po[j][:, :nsz],
                                lhsT=xtb[:, c, it * P:(it + 1) * P],
                                rhs=wt[:, c, n0:n0 + nsz],
                                start=(c == 0), stop=(c == NK - 1))
                    osb = outp.tile([P, out_sh], dt.float32, tag="osb")
                    for j, (n0, nsz) in enumerate(nsl):
                        nc.vector.tensor_add(osb[:, n0:n0 + nsz],
                                             po[j][:, :nsz],
                                             bias_bc[:, n0:n0 + nsz])
                    nc.scalar.dma_start(out[t0:t0 + P, :], osb[:])

    nc.compile()
    return nc


def make_in_maps(x, packed, scales, offsets, inv_scale, bias, out_sh=OUT_SH):
    n_o_tiles = (out_sh + P - 1) // P
    out_pad = n_o_tiles * P
    xf = np.asarray(x, dtype=np.float32).reshape(-1, IN)
    xth = np.ascontiguousarray(xf.T)
    pkm = np.asarray(packed, dtype=np.int32).reshape(OUT, IN // 2)
    scm = np.asarray(scales, dtype=np.float32).reshape(OUT, NK)
    ofm = np.asarray(offsets, dtype=np.float32).reshape(OUT, NK)
    invv = np.ascontiguousarray(
        np.asarray(inv_scale, dtype=np.float32).reshape(NK, P).T)
    bv = np.asarray(bias, dtype=np.float32)
    pad = out_pad - out_sh
    in_maps = []
    for k in range(N_CORES):
        sl = slice(k * out_sh, (k + 1) * out_sh)
        pk_k = np.pad(pkm[sl], ((0, pad), (0, 0)))
        # [out_pad, IN//2] -> [NK, 128, n_o_tiles*64]
        pk_k = pk_k.reshape(n_o_tiles, P, NK, 64).transpose(2, 1, 0, 3)
        pk_k = np.ascontiguousarray(pk_k.reshape(NK, P, n_o_tiles * 64))
        sc_k = np.pad(scm[sl], ((0, pad), (0, 0)), constant_values=1.0)
        sc_k = np.ascontiguousarray(
            sc_k.reshape(n_o_tiles, P, NK).transpose(1, 0, 2))
        of_k = np.pad(ofm[sl], ((0, pad), (0, 0)))
        of_k = np.ascontiguousarray(
            of_k.reshape(n_o_tiles, P, NK).transpose(1, 0, 2))
        in_maps.append({
            "xt": xth,
            "pk": pk_k,
            "sc": sc_k,
            "of": of_k,
            "inv": invv,
            "bias": np.ascontiguousarray(bv[sl]),
        })
    return in_maps


_CACHE = {}


def kernel(x, packed, scales, offsets, inv_scale, bias):
    if "nc" not in _CACHE:
        _CACHE["nc"] = build()
    nc = _CACHE["nc"]
    in_maps = make_in_maps(x, packed, scales, offsets, inv_scale, bias)
    res = run_bass_kernel_spmd(nc, in_maps, list(range(N_CORES)))
    cols = [res.results[k]["out"] for k in range(N_CORES)]
    full = np.concatenate(cols, axis=1)
    return np.ascontiguousarray(full.reshape(4, 2048, OUT).astype(np.float32))


# revision 17
# speedup vs baseline: 1.2069x; 1.2069x over previous
"""AWQ quantized linear (4096 -> 11008) on 8 trn2 NeuronCores.

Column-parallel sharding: each core owns OUT/8 = 1376 output features.
Host side does only sharding + index-permutation (no arithmetic): the packed
int32 words are resharded chunk-major ([NK, 128, 11*64] per core, padded to
11 full 128-row o-tiles) so every device DMA is contiguous per partition.

Per core (all compute on device):
  - unpack int32-packed nibbles (low/high interleaved) on DVE
  - per-group (128-wide) affine dequant via tensor_scalar (per-partition
    scalars), chunk-outer so wt[:, c, :] completes incrementally
  - PE-transpose weight tiles to [IN, OUT_SH] fp16 resident in SBUF, folding
    the per-input-channel inv_scale into the PSUM->SBUF copy
  - x path: SWDGE cast-DMA f32->fp16 into per-block internal DRAM tiles, then
    hardware DMA-transpose (xbar) loads xT k-chunk tiles straight into SBUF
  - DMA queue separation: casts on gpsimd (SWDGE), xbar transposes on sync
    (HWDGE-SP), everything else on scalar (HWDGE-ACT)
  - fp16 matmuls accumulate in f32 PSUM over 32 K-chunks; N-slices 512/512/352
  - bias added during PSUM->SBUF copy; f32 stores
"""

import sys

for _p in ("/opt/trn_rl_repo", "/opt/pypackages"):
    if _p not in sys.path:
        sys.path.append(_p)

import numpy as np

import concourse.bass as bass
import concourse.mybir as mybir
import concourse.tile as tile
from concourse import bacc
from concourse.bass_utils import run_bass_kernel_spmd
from concourse.masks import make_identity

IN = 4096
OUT = 11008
N_CORES = 8
OUT_SH = OUT // N_CORES  # 1376
T = 8192
NK = IN // 128  # 32 k-chunks
P = 128
TB = 512       # token block for DMA-transpose staging

dt = mybir.dt
Alu = mybir.AluOpType
Act = mybir.ActivationFunctionType


def build(n_t_tiles=T // P, out_sh=OUT_SH):
    n_o_tiles = (out_sh + P - 1) // P          # 11
    out_pad = n_o_tiles * P                    # 1408 (wt padded, output not)
    nsl = []
    n0 = 0
    while n0 < out_sh:
        nsz = min(512, out_sh - n0)
        nsl.append((n0, nsz))
        n0 += nsz

    n_tok = n_t_tiles * P
    tb = min(TB, n_tok)
    n_blocks = (n_tok + tb - 1) // tb
    tiles_per_block = tb // P

    nc = bacc.Bacc("TRN2", target_bir_lowering=False, debug=False,
                   num_devices=N_CORES)
    xt = nc.dram_tensor("xt", [IN, n_tok], dt.float32,
                        kind="ExternalInput").ap()
    # host-repacked: pk_h[c, p, ot*64+f] = packed[(k*OUT_SH)+ot*128+p, c*64+f]
    pk = nc.dram_tensor("pk", [NK, P, n_o_tiles * 64], dt.int32,
                        kind="ExternalInput").ap()
    sc = nc.dram_tensor("sc", [P, n_o_tiles, NK], dt.float32,
                        kind="ExternalInput").ap()
    of = nc.dram_tensor("of", [P, n_o_tiles, NK], dt.float32,
                        kind="ExternalInput").ap()
    inv = nc.dram_tensor("inv", [P, NK], dt.float32,
                         kind="ExternalInput").ap()
    bias = nc.dram_tensor("bias", [out_sh], dt.float32,
                          kind="ExternalInput").ap()
    out = nc.dram_tensor("out", [n_tok, out_sh], dt.float32,
                         kind="ExternalOutput").ap()

    with tile.TileContext(nc) as tc:
        with (
            tc.tile_pool(name="const", bufs=1) as constp,
            tc.tile_pool(name="wtp", bufs=1) as wtp,
            tc.tile_pool(name="prep", bufs=2) as prep,
            tc.tile_pool(name="prepsm", bufs=2) as prepsm,
            tc.tile_pool(name="xtp", bufs=2) as xtp,
            tc.tile_pool(name="outp", bufs=2) as outp,
            tc.tile_pool(name="pmm", bufs=2 * len(nsl), space="PSUM") as pmm,
            tc.tile_pool(name="ptp", bufs=2, space="PSUM") as ptp,
        ):
            ident16 = constp.tile([P, P], dt.float16)
            make_identity(nc, ident16[:])

            inv_sb = constp.tile([P, NK], dt.float32)
            nc.scalar.dma_start(inv_sb[:], inv[:])

            bias_bc = constp.tile([P, out_sh], dt.float16)
            nc.gpsimd.dma_start(bias_bc[:],
                                bias[None, :].to_broadcast([P, out_sh]))

            # fp16 W^T, laid out [128 (i within chunk), NK chunks, out_pad]
            wt = wtp.tile([P, NK, out_pad], dt.float16)

            # ---- weight prep (chunk-outer; all loads contiguous) ----
            sc_all = constp.tile([P, n_o_tiles, NK], dt.float32)
            of_all = constp.tile([P, n_o_tiles, NK], dt.float32)
            nc.scalar.dma_start(sc_all[:], sc[:])
            nc.scalar.dma_start(of_all[:], of[:])

            for c in range(NK):
                pkt = prep.tile([P, n_o_tiles * 64], dt.int32, tag="pkt",
                                name=f"pkt{c}")
                nc.scalar.dma_start(pkt[:], pk[c])
                wq = prepsm.tile([P, n_o_tiles * 64, 2], dt.int32, tag="wq",
                                 name=f"wq{c}")
                nc.vector.tensor_scalar(wq[:, :, 0], pkt[:], 15, None,
                                        op0=Alu.bitwise_and)
                nc.vector.tensor_scalar(wq[:, :, 1], pkt[:], 4, None,
                                        op0=Alu.logical_shift_right)
                for ot in range(n_o_tiles):
                    o0 = ot * P
                    wd = prepsm.tile([P, P], dt.float16, tag="wd")
                    nc.vector.tensor_scalar(
                        wd[:],
                        wq[:, ot * 64:(ot + 1) * 64, :].rearrange(
                            "p a b -> p (a b)"),
                        sc_all[:, ot, c:c + 1], of_all[:, ot, c:c + 1],
                        op0=Alu.mult, op1=Alu.add)
                    ps = ptp.tile([P, P], dt.float32, tag="tp")
                    # transpose as a normal fp16 matmul against identity:
                    # exact (x*1.0 into f32 PSUM) and keeps the HAM clock warm
                    nc.tensor.matmul(ps[:], lhsT=wd[:], rhs=ident16[:],
                                     start=True, stop=True)
                    if ot % 2 == 0:
                        nc.scalar.activation(wt[:, c, o0:o0 + P], ps[:],
                                             Act.Copy,
                                             scale=inv_sb[:, c:c + 1])
                    else:
                        nc.vector.tensor_scalar(wt[:, c, o0:o0 + P], ps[:],
                                                inv_sb[:, c:c + 1], None,
                                                op0=Alu.mult)

            # ---- main loop over token blocks ----
            for b in range(n_blocks):
                xtb = xtp.tile([P, NK, tb], dt.float16, tag="xtb",
                               name=f"xtb{b}")
                for c in range(NK):
                    nc.gpsimd.dma_start(
                        xtb[:, c, :],
                        xt[c * P:(c + 1) * P, b * tb:(b + 1) * tb])
                for it in range(tiles_per_block):
                    tt = b * tiles_per_block + it
                    t0 = tt * P
                    po = [pmm.tile([P, 512], dt.float32, tag="po",
                                   name=f"po{tt}_{j}")
                          for j in range(len(nsl))]
                    for c in range(NK):
                        for j, (n0, nsz) in enumerate(nsl):
                            nc.tensor.matmul(
                                po[j][:, :nsz],
                                lhsT=xtb[:, c, it * P:(it + 1) * P],
                                rhs=wt[:, c, n0:n0 + nsz],
                                start=(c == 0), stop=(c == NK - 1))
                    osb = outp.tile([P, out_sh], dt.float32, tag="osb")
                    for j, (n0, nsz) in enumerate(nsl):
                        nc.vector.tensor_add(osb[:, n0:n0 + nsz],
                                             po[j][:, :nsz],
                                             bias_bc[:, n0:n0 + nsz])
                    nc.scalar.dma_start(out[t0:t0 + P, :], osb[:])

    nc.compile()
    return nc


def make_in_maps(x, packed, scales, offsets, inv_scale, bias, out_sh=OUT_SH):
    n_o_tiles = (out_sh + P - 1) // P
    out_pad = n_o_tiles * P
    xf = np.asarray(x, dtype=np.float32).reshape(-1, IN)
    xth = np.ascontiguousarray(xf.T)
    pkm = np.asarray(packed, dtype=np.int32).reshape(OUT, IN // 2)
    scm = np.asarray(scales, dtype=np.float32).reshape(OUT, NK)
    ofm = np.asarray(offsets, dtype=np.float32).reshape(OUT, NK)
    invv = np.ascontiguousarray(
        np.asarray(inv_scale, dtype=np.float32).reshape(NK, P).T)
    bv = np.asarray(bias, dtype=np.float32)
    pad = out_pad - out_sh
    in_maps = []
    for k in range(N_CORES):
        sl = slice(k * out_sh, (k + 1) * out_sh)
        pk_k = np.pad(pkm[sl], ((0, pad), (0, 0)))
        # [out_pad, IN//2] -> [NK, 128, n_o_tiles*64]
        pk_k = pk_k.reshape(n_o_tiles, P, NK, 64).transpose(2, 1, 0, 3)
        pk_k = np.ascontiguousarray(pk_k.reshape(NK, P, n_o_tiles * 64))
        sc_k = np.pad(scm[sl], ((0, pad), (0, 0)), constant_values=1.0)
        sc_k = np.ascontiguousarray(
            sc_k.reshape(n_o_tiles, P, NK).transpose(1, 0, 2))
        of_k = np.pad(ofm[sl], ((0, pad), (0, 0)))
        of_k = np.ascontiguousarray(
            of_k.reshape(n_o_tiles, P, NK).transpose(1, 0, 2))
        in_maps.append({
            "xt": xth,
            "pk": pk_k,
            "sc": sc_k,
            "of": of_k,
            "inv": invv,
            "bias": np.ascontiguousarray(bv[sl]),
        })
    return in_maps


_CACHE = {}


def kernel(x, packed, scales, offsets, inv_scale, bias):
    if "nc" not in _CACHE:
        _CACHE["nc"] = build()
    nc = _CACHE["nc"]
    in_maps = make_in_maps(x, packed, scales, offsets, inv_scale, bias)
    res = run_bass_kernel_spmd(nc, in_maps, list(range(N_CORES)))
    cols = [res.results[k]["out"] for k in range(N_CORES)]
    full = np.concatenate(cols, axis=1)
    return np.ascontiguousarray(full.reshape(4, 2048, OUT).astype(np.float32))
